# Initial kernel scaffold
#
# Swin-style window attention (B=256 windows, N=196, C=768, H=12) on 8 trn2 cores.
# Data-parallel over windows: 32 windows/core. Inside each core:
#   qkvT = W_qkv @ x.T        (fp32r GEMMs, x pre-transposed on host)
#   per (window, head):  S.T = K.T.T@Q.T -> exp -> *exp(rpb).T -> colsum via
#   ones-matmul (pre-broadcast) -> 1/x -> O.T = (V+vb).T@P.T * recip
#   yT = W_proj @ O.T + b     (fp32r GEMMs)
# Zero on-device transposes: all orientations arranged so matmul contraction
# dims land on partitions naturally.
import sys

sys.path.insert(0, "/opt/trn_rl_repo")

from contextlib import ExitStack

import ml_dtypes
import numpy as np

import concourse.bacc as bacc
import concourse.mybir as mybir
import concourse.tile as tile
from concourse.bass_utils import run_bass_kernel_spmd

F32 = mybir.dt.float32
F32R = mybir.dt.float32r
BF16 = mybir.dt.bfloat16
AF = mybir.ActivationFunctionType

NCORES = 8
B, N, C, H = 256, 196, 768, 12
HD = C // H  # 64
WPC = B // NCORES  # 32 windows per core
T = WPC * N  # 6272 tokens per core
CB = C // 128  # 6 contraction blocks
CHUNK_W = 4  # windows per chunk
CHUNK_T = CHUNK_W * N  # 784 tokens
NCHUNK = WPC // CHUNK_W  # 8


def _build_nc(wpc=WPC, chunk_w=CHUNK_W):
    t_total = wpc * N
    nchunk = wpc // chunk_w
    chunk_t = chunk_w * N

    nc = bacc.Bacc("TRN2", target_bir_lowering=False, debug=False,
                   num_devices=NCORES)
    xT_d = nc.dram_tensor("xT", [C, t_total], F32, kind="ExternalInput").ap()
    wq_d = nc.dram_tensor("qkvwT", [C, 3 * C], F32, kind="ExternalInput").ap()
    wp_d = nc.dram_tensor("projwT", [C, C], F32, kind="ExternalInput").ap()
    qb_d = nc.dram_tensor("qbT", [128, CB], F32, kind="ExternalInput").ap()
    pb_d = nc.dram_tensor("pbT", [128, CB], F32, kind="ExternalInput").ap()
    vb_d = nc.dram_tensor("vb", [1, C], F32, kind="ExternalInput").ap()
    erp_d = nc.dram_tensor("erpT", [H, N, N], BF16, kind="ExternalInput").ap()
    yT_d = nc.dram_tensor("yT", [C, t_total], F32, kind="ExternalOutput").ap()

    with tile.TileContext(nc) as tc, ExitStack() as ctx:
        const = ctx.enter_context(tc.tile_pool(name="const", bufs=1))
        wpool = ctx.enter_context(tc.tile_pool(name="w", bufs=1))
        xpool = ctx.enter_context(tc.tile_pool(name="x", bufs=2))
        qkpool = ctx.enter_context(tc.tile_pool(name="qk", bufs=1))
        vpool = ctx.enter_context(tc.tile_pool(name="v", bufs=1))
        otpool = ctx.enter_context(tc.tile_pool(name="ot", bufs=1))
        ppool = ctx.enter_context(tc.tile_pool(name="p", bufs=4))
        rpool = ctx.enter_context(tc.tile_pool(name="r", bufs=3))
        ypool = ctx.enter_context(tc.tile_pool(name="y", bufs=3))
        ps_mm = ctx.enter_context(tc.tile_pool(name="psmm", bufs=2, space="PSUM"))
        ps_v = ctx.enter_context(tc.tile_pool(name="psv", bufs=1, space="PSUM"))
        ps_st = ctx.enter_context(tc.tile_pool(name="psst", bufs=2, space="PSUM"))
        ps_co = ctx.enter_context(tc.tile_pool(name="psco", bufs=2, space="PSUM"))

        # ---- resident constants / weights ----
        wq = []
        for cb in range(CB):
            t = wpool.tile([128, 3 * C], F32, tag=f"wq{cb}")
            nc.sync.dma_start(t[:], wq_d[cb * 128:(cb + 1) * 128, :])
            wq.append(t)
        wp = []
        for cb in range(CB):
            t = wpool.tile([128, C], F32, tag=f"wp{cb}")
            nc.sync.dma_start(t[:], wp_d[cb * 128:(cb + 1) * 128, :])
            wp.append(t)
        erp = []
        for h in range(H):
            t0 = wpool.tile([128, N], BF16, tag=f"erp{h}a")
            t1 = wpool.tile([68, N], BF16, tag=f"erp{h}b")
            nc.sync.dma_start(t0[:], erp_d[h, 0:128, :])
            nc.sync.dma_start(t1[:], erp_d[h, 128:196, :])
            erp.append((t0, t1))
        ones64 = const.tile([128, HD], BF16)
        nc.vector.memset(ones64[:], 1.0)
        onesrow = const.tile([1, 128], F32)
        nc.vector.memset(onesrow[:], 1.0)
        qb = const.tile([128, CB], F32)
        nc.sync.dma_start(qb[:], qb_d[:, :])
        pb = const.tile([128, CB], F32)
        nc.sync.dma_start(pb[:], pb_d[:, :])
        vb = const.tile([1, C], F32)
        nc.sync.dma_start(vb[:], vb_d[:, :])

        for ch in range(nchunk):
            t0 = ch * chunk_t
            # ---- load x.T chunk ----
            xt = []
            for cb in range(CB):
                t = xpool.tile([128, chunk_t], F32, tag=f"xt{cb}")
                nc.sync.dma_start(t[:], xT_d[cb * 128:(cb + 1) * 128,
                                             t0:t0 + chunk_t])
                xt.append(t)

            # token sub-tiles for the N-dim of the QK/proj GEMMs
            tslices = [(i * 512, min(512, chunk_t - i * 512))
                       for i in range((chunk_t + 511) // 512)]

            # ---- Q.T / K.T : psum[o128, t] = sum_cb wq[cb][:, o].T @ xt[cb] ----
            qT, kT = [], []
            for ob in range(CB):
                t = qkpool.tile([128, chunk_t], BF16, tag=f"qT{ob}")
                qT.append(t)
            for ob in range(CB):
                t = qkpool.tile([128, chunk_t], BF16, tag=f"kT{ob}")
                kT.append(t)
            for ob in range(2 * CB):
                dst = qT[ob] if ob < CB else kT[ob - CB]
                o = ob * 128
                for (ts, tl) in tslices:
                    pt = ps_mm.tile([128, 512], F32, tag="mm")
                    for cb in range(CB):
                        nc.tensor.matmul(
                            pt[:, 0:tl],
                            wq[cb][:, o:o + 128].bitcast(F32R),
                            xt[cb][:, ts:ts + tl].bitcast(F32R),
                            start=(cb == 0), stop=(cb == CB - 1))
                    if ob < CB:  # q: add bias (scale folded into exp later)
                        nc.scalar.activation(dst[:, ts:ts + tl], pt[:, 0:tl],
                                             AF.Identity,
                                             bias=qb[:, ob:ob + 1])
                    else:  # k: plain copy/cast
                        nc.scalar.copy(dst[:, ts:ts + tl], pt[:, 0:tl])

            # ---- V (+vb) token-major: psum[t, o] per window-aligned block ----
            vtiles = []
            for w in range(chunk_w):
                wrow = []
                for (moff, mlen) in ((0, 128), (128, 68)):
                    trel = w * N + moff
                    vt = vpool.tile([128, C], BF16, tag=f"vb{w}_{moff}")
                    pv = ps_v.tile([128, C], F32, tag="v")
                    for (noff, nlen) in ((0, 512), (512, 256)):
                        nc.tensor.matmul(
                            pv[0:mlen, noff:noff + nlen],
                            onesrow[:, 0:mlen].bitcast(F32R),
                            vb[:, 2 * C + noff - 2 * C + noff:][:, 0:0] if False
                            else vb[:, noff:noff + nlen].bitcast(F32R),
                            start=True, stop=False)
                        for cb in range(CB):
                            nc.tensor.matmul(
                                pv[0:mlen, noff:noff + nlen],
                                xt[cb][:, trel:trel + mlen].bitcast(F32R),
                                wq[cb][:, 2 * C + noff:2 * C + noff + nlen]
                                .bitcast(F32R),
                                start=False, stop=(cb == CB - 1))
                    nc.vector.tensor_copy(vt[0:mlen, :], pv[0:mlen, :])
                    wrow.append(vt)
                vtiles.append(wrow)

            # ---- attention per (window, head) ----
            ot_sb = []
            for ob in range(CB):
                t = otpool.tile([128, chunk_t], F32, tag=f"ot{ob}")
                ot_sb.append(t)
            for w in range(chunk_w):
                wq_tok = w * N
                for h in range(H):
                    ob, prt = h // 2, (h % 2) * 64
                    qh = qT[ob][prt:prt + 64, wq_tok:wq_tok + N]
                    st = ps_st.tile([128, 2 * N], F32, tag="st")
                    nc.tensor.matmul(
                        st[:, 0:N],
                        kT[ob][prt:prt + 64, wq_tok:wq_tok + 128],
                        qh, start=True, stop=True)
                    nc.tensor.matmul(
                        st[0:68, N:2 * N],
                        kT[ob][prt:prt + 64, wq_tok + 128:wq_tok + N],
                        qh, start=True, stop=True)
                    p0 = ppool.tile([128, N], BF16, tag="p0")
                    p1 = ppool.tile([68, N], BF16, tag="p1")
                    nc.scalar.activation(p0[:], st[:, 0:N], AF.Exp, scale=0.125)
                    nc.scalar.activation(p1[:], st[0:68, N:2 * N], AF.Exp,
                                         scale=0.125)
                    nc.vector.tensor_mul(p0[:], p0[:], erp[h][0][:])
                    nc.vector.tensor_mul(p1[:], p1[:], erp[h][1][:])
                    cs = ps_co.tile([64, N], F32, tag="cs")
                    nc.tensor.matmul(cs[:], ones64[:, :], p0[:],
                                     start=True, stop=False)
                    nc.tensor.matmul(cs[:], ones64[0:68, :], p1[:],
                                     start=False, stop=True)
                    rec = rpool.tile([64, N], F32, tag="rec")
                    nc.vector.reciprocal_approx_fast(rec[:], cs[:])
                    ot = ps_co.tile([64, N], F32, tag="ot")
                    nc.tensor.matmul(ot[:], vtiles[w][0][:, h * 64:h * 64 + 64],
                                     p0[:], start=True, stop=False)
                    nc.tensor.matmul(ot[:], vtiles[w][1][0:68,
                                                         h * 64:h * 64 + 64],
                                     p1[:], start=False, stop=True)
                    nc.vector.tensor_mul(
                        ot_sb[ob][prt:prt + 64, wq_tok:wq_tok + N],
                        ot[:], rec[:])

            # ---- proj: yT[o'] = sum_ob wp[ob][:, o'].T @ ot_sb[ob] + pb ----
            for opb in range(CB):
                o = opb * 128
                for (ts, tl) in tslices:
                    pt = ps_mm.tile([128, 512], F32, tag="mm")
                    for ob in range(CB):
                        nc.tensor.matmul(
                            pt[:, 0:tl],
                            wp[ob][:, o:o + 128].bitcast(F32R),
                            ot_sb[ob][:, ts:ts + tl].bitcast(F32R),
                            start=(ob == 0), stop=(ob == CB - 1))
                    yt = ypool.tile([128, 512], F32, tag="y")
                    nc.scalar.activation(yt[:, 0:tl], pt[:, 0:tl], AF.Identity,
                                         bias=pb[:, opb:opb + 1])
                    nc.sync.dma_start(yT_d[o:o + 128, t0 + ts:t0 + ts + tl],
                                      yt[:, 0:tl])

    nc.compile()
    return nc


def _host_prep(x, qkv_w, q_bias, v_bias, rpb_table, proj_w, proj_b, rel_index,
               wpc=WPC):
    x = np.asarray(x, np.float32)
    ncores = x.shape[0] // wpc
    t_total = wpc * N
    xT = np.ascontiguousarray(
        x.reshape(ncores, t_total, C).transpose(0, 2, 1))
    qkvwT = np.ascontiguousarray(np.asarray(qkv_w, np.float32).T)
    projwT = np.ascontiguousarray(np.asarray(proj_w, np.float32).T)
    qbT = np.ascontiguousarray(
        np.asarray(q_bias, np.float32).reshape(CB, 128).T)
    pbT = np.ascontiguousarray(
        np.asarray(proj_b, np.float32).reshape(CB, 128).T)
    vb = np.asarray(v_bias, np.float32).reshape(1, C)
    rel = np.asarray(rel_index).reshape(N, N)
    rpb = np.asarray(rpb_table, np.float32)[rel]          # [n, m, H]
    erpT = np.ascontiguousarray(
        np.exp(rpb).transpose(2, 1, 0)).astype(ml_dtypes.bfloat16)  # [H, m, n]
    return xT, qkvwT, projwT, qbT, pbT, vb, erpT


def kernel(x, qkv_w, q_bias, v_bias, rpb_table, proj_w, proj_b, rel_index,
           num_heads=12, _trace=False):
    xT, qkvwT, projwT, qbT, pbT, vb, erpT = _host_prep(
        x, qkv_w, q_bias, v_bias, rpb_table, proj_w, proj_b, rel_index)
    nc = _build_nc()
    in_maps = [
        {"xT": np.ascontiguousarray(xT[c]), "qkvwT": qkvwT, "projwT": projwT,
         "qbT": qbT, "pbT": pbT, "vb": vb, "erpT": erpT}
        for c in range(NCORES)
    ]
    res = run_bass_kernel_spmd(nc, in_maps, core_ids=list(range(NCORES)),
                               trace=_trace)
    yT = np.stack([res.results[c]["yT"] for c in range(NCORES)])
    out = np.ascontiguousarray(yT.transpose(0, 2, 1)).reshape(B, N, C)
    if _trace:
        kernel._last_exec_time_ns = res.exec_time_ns
        kernel._last_results = res
    return out.astype(np.float32)


# revision 13
# speedup vs baseline: 1.0077x; 1.0077x over previous
# Swin-style window attention (B=256 windows, N=196, C=768, H=12) on 8 trn2 cores.
# Data-parallel over windows: 32 windows/core. Inside each core:
#   qkvT = W_qkv @ x.T        (fp32r GEMMs, x pre-transposed on host)
#   per (window, head):  S.T = K.T.T@Q.T -> exp -> *exp(rpb).T -> colsum via
#   ones-matmul (pre-broadcast) -> 1/x -> O.T = (V+vb).T@P.T * recip
#   yT = W_proj @ O.T + b     (fp32r GEMMs)
# Zero on-device transposes: all orientations arranged so matmul contraction
# dims land on partitions naturally.
import sys

sys.path.insert(0, "/opt/trn_rl_repo")

from contextlib import ExitStack

import ml_dtypes
import numpy as np

import concourse.bacc as bacc
import concourse.mybir as mybir
import concourse.tile as tile
from concourse.bass_utils import run_bass_kernel_spmd

F32 = mybir.dt.float32
F32R = mybir.dt.float32r
BF16 = mybir.dt.bfloat16
AF = mybir.ActivationFunctionType


def _install_ntff_hook():
    """Recreate the antenv.axon_hooks shim so trace=True works under axon."""
    import types

    if "antenv.axon_hooks" in sys.modules:
        return
    mod = types.ModuleType("antenv.axon_hooks")
    mod._hook = None
    mod.set_axon_ntff_profile_hook = lambda h: setattr(mod, "_hook", h)
    mod.get_axon_ntff_profile_hook = lambda: mod._hook
    sys.modules["antenv.axon_hooks"] = mod
    try:
        sys.path.insert(0, "/root/.axon_site/trn_agent_boot")
        from trn_boot import _ntff_profile_via_ctypes

        hook = _ntff_profile_via_ctypes("/opt/axon/libaxon_pjrt.so")
        if hook is not None:
            mod._hook = hook
    except Exception:
        pass

_NC_CACHE = {}
NCORES = 8
B, N, C, H = 256, 196, 768, 12
HD = C // H  # 64
WPC = B // NCORES  # 32 windows per core
T = WPC * N  # 6272 tokens per core
CB = C // 128  # 6 contraction blocks
CHUNK_W = 4  # windows per chunk
CHUNK_T = CHUNK_W * N  # 784 tokens
NCHUNK = WPC // CHUNK_W  # 8


def _build_nc(wpc=WPC, chunk_w=CHUNK_W):
    t_total = wpc * N
    nchunk = wpc // chunk_w
    chunk_t = chunk_w * N

    nc = bacc.Bacc("TRN2", target_bir_lowering=False, debug=False,
                   num_devices=NCORES)
    xT_d = nc.dram_tensor("xT", [C, t_total], F32R, kind="ExternalInput").ap()
    wq_d = nc.dram_tensor("qkvwT", [C, 3 * C], F32R, kind="ExternalInput").ap()
    wp_d = nc.dram_tensor("projwT", [C, C], F32R, kind="ExternalInput").ap()
    qb_d = nc.dram_tensor("qbT", [128, CB], F32, kind="ExternalInput").ap()
    pb_d = nc.dram_tensor("pbT", [128, CB], F32, kind="ExternalInput").ap()
    vb_d = nc.dram_tensor("vb", [1, C], F32R, kind="ExternalInput").ap()
    ones_d = nc.dram_tensor("onesrow", [1, 128], F32R,
                            kind="ExternalInput").ap()
    erp_d = nc.dram_tensor("erpT", [H, 128, 2 * N], BF16, kind="ExternalInput").ap()
    yT_d = nc.dram_tensor("yT", [C, t_total], F32, kind="ExternalOutput").ap()

    with tile.TileContext(nc) as tc, ExitStack() as ctx:
        const = ctx.enter_context(tc.tile_pool(name="const", bufs=1))
        wpool = ctx.enter_context(tc.tile_pool(name="w", bufs=1))
        xpool = ctx.enter_context(tc.tile_pool(name="x", bufs=2))
        qkpool = ctx.enter_context(tc.tile_pool(name="qk", bufs=1))
        vpool = ctx.enter_context(tc.tile_pool(name="v", bufs=1))
        otpool = ctx.enter_context(tc.tile_pool(name="ot", bufs=1))
        ppool = ctx.enter_context(tc.tile_pool(name="p", bufs=4))
        rpool = ctx.enter_context(tc.tile_pool(name="r", bufs=3))
        ypool = ctx.enter_context(tc.tile_pool(name="y", bufs=3))
        ps_mm = ctx.enter_context(tc.tile_pool(name="psmm", bufs=2, space="PSUM"))
        ps_st = ctx.enter_context(tc.tile_pool(name="psst", bufs=2, space="PSUM"))
        ps_co = ctx.enter_context(tc.tile_pool(name="psco", bufs=1, space="PSUM"))

        # ---- resident constants / weights ----
        wq = []
        for cb in range(CB):
            t = wpool.tile([128, 3 * C], F32R, tag=f"wq{cb}")
            nc.sync.dma_start(t[:], wq_d[cb * 128:(cb + 1) * 128, :])
            wq.append(t)
        wp = []
        for cb in range(CB):
            t = wpool.tile([128, C], F32R, tag=f"wp{cb}")
            nc.sync.dma_start(t[:], wp_d[cb * 128:(cb + 1) * 128, :])
            wp.append(t)
        erp = []
        for h in range(H):
            t = wpool.tile([128, 2 * N], BF16, tag=f"erp{h}")
            nc.sync.dma_start(t[:], erp_d[h, :, :])
            erp.append(t)
        ones64 = const.tile([128, HD], BF16)
        nc.vector.memset(ones64[:], 1.0)
        qb = const.tile([128, CB], F32)
        nc.sync.dma_start(qb[:], qb_d[:, :])
        pb = const.tile([128, CB], F32)
        nc.sync.dma_start(pb[:], pb_d[:, :])
        vb = const.tile([1, C], F32R)
        nc.sync.dma_start(vb[:], vb_d[:, :])
        onesrow = const.tile([1, 128], F32R)
        nc.sync.dma_start(onesrow[:], ones_d[:, :])

        for ch in range(nchunk):
            t0 = ch * chunk_t
            # ---- load x.T chunk ----
            xt = []
            for cb in range(CB):
                t = xpool.tile([128, chunk_t], F32R, tag=f"xt{cb}")
                nc.sync.dma_start(t[:], xT_d[cb * 128:(cb + 1) * 128,
                                             t0:t0 + chunk_t])
                xt.append(t)

            # token sub-tiles for the N-dim of the QK/proj GEMMs
            tslices = [(i * 512, min(512, chunk_t - i * 512))
                       for i in range((chunk_t + 511) // 512)]

            # ---- Q.T / K.T : psum[o128, t] = sum_cb wq[cb][:, o].T @ xt[cb] ----
            qT, kT = [], []
            for ob in range(CB):
                t = qkpool.tile([128, chunk_t], BF16, tag=f"qT{ob}")
                qT.append(t)
            for ob in range(CB):
                t = qkpool.tile([128, chunk_t + 64], BF16, tag=f"kT{ob}")
                nc.vector.memset(t[:, chunk_t:chunk_t + 64], 0.0)
                kT.append(t)
            for ob in range(2 * CB):
                dst = qT[ob] if ob < CB else kT[ob - CB]
                o = ob * 128
                pt = ps_mm.tile([128, chunk_t], F32, tag="mm")
                for (ts, tl) in tslices:
                    for cb in range(CB):
                        nc.tensor.matmul(
                            pt[:, ts:ts + tl],
                            wq[cb][:, o:o + 128],
                            xt[cb][:, ts:ts + tl],
                            start=(cb == 0), stop=(cb == CB - 1))
                if ob < CB:  # q: add bias (scale folded into exp later)
                    nc.scalar.activation(dst[:], pt[:], AF.Identity,
                                         bias=qb[:, ob:ob + 1])
                else:  # k: plain copy/cast
                    nc.scalar.copy(dst[:, 0:chunk_t], pt[:])

            # ---- V (+vb) token-major: psum[t, o] per window-aligned block ----
            vtiles = []
            for w in range(chunk_w):
                wrow = []
                for (moff, mlen) in ((0, 128), (128, 68)):
                    trel = w * N + moff
                    vt = vpool.tile([128, C], BF16, tag=f"vb{w}_{moff}")
                    pv = ps_mm.tile([128, chunk_t], F32, tag="mm")
                    for (noff, nlen) in ((0, 512), (512, 256)):
                        nc.tensor.matmul(
                            pv[0:mlen, noff:noff + nlen],
                            onesrow[:, 0:mlen],
                            vb[:, noff:noff + nlen],
                            start=True, stop=False)
                        for cb in range(CB):
                            nc.tensor.matmul(
                                pv[0:mlen, noff:noff + nlen],
                                xt[cb][:, trel:trel + mlen],
                                wq[cb][:, 2 * C + noff:2 * C + noff + nlen]
                                ,
                                start=False, stop=(cb == CB - 1))
                    nc.vector.tensor_copy(vt[0:mlen, :], pv[0:mlen, 0:C])
                    wrow.append(vt)
                vtiles.append(wrow)

            # ---- attention per (window, head) ----
            ot_sb = []
            for ob in range(CB):
                t = otpool.tile([128, chunk_t], F32R, tag=f"ot{ob}")
                ot_sb.append(t)
            for w in range(chunk_w):
                wq_tok = w * N
                for h in range(H):
                    ob, prt = h // 2, (h % 2) * 64
                    qh = qT[ob][prt:prt + 64, wq_tok:wq_tok + N]
                    st = ps_st.tile([128, 2 * N], F32, tag="st")
                    nc.tensor.matmul(
                        st[:, 0:N],
                        kT[ob][prt:prt + 64, wq_tok:wq_tok + 128],
                        qh, start=True, stop=True)
                    nc.tensor.matmul(
                        st[:, N:2 * N],
                        kT[ob][prt:prt + 64, wq_tok + 128:wq_tok + 256],
                        qh, start=True, stop=True)
                    p = ppool.tile([128, 2 * N], BF16, tag="p")
                    nc.scalar.activation(p[:], st[:], AF.Exp, scale=0.125)
                    nc.vector.tensor_mul(p[:], p[:], erp[h][:])
                    p0 = p[:, 0:N]
                    p1 = p[0:68, N:2 * N]
                    cs = ps_co.tile([64, N], F32, tag="cs")
                    nc.tensor.matmul(cs[:], ones64[:, :], p0,
                                     start=True, stop=False)
                    nc.tensor.matmul(cs[:], ones64[0:68, :], p1,
                                     start=False, stop=True)
                    rec = rpool.tile([64, N], F32, tag="rec")
                    nc.vector.reciprocal_approx_fast(rec[:], cs[:])
                    ot = ps_co.tile([64, N], F32, tag="ot")
                    nc.tensor.matmul(ot[:], vtiles[w][0][:, h * 64:h * 64 + 64],
                                     p0, start=True, stop=False)
                    nc.tensor.matmul(ot[:], vtiles[w][1][0:68,
                                                         h * 64:h * 64 + 64],
                                     p1, start=False, stop=True)
                    nc.vector.tensor_mul(
                        ot_sb[ob][prt:prt + 64, wq_tok:wq_tok + N],
                        ot[:], rec[:])

            # ---- proj: yT[o'] = sum_ob wp[ob][:, o'].T @ ot_sb[ob] + pb ----
            for opb in range(CB):
                o = opb * 128
                pt = ps_mm.tile([128, chunk_t], F32, tag="mm")
                for (ts, tl) in tslices:
                    for ob in range(CB):
                        nc.tensor.matmul(
                            pt[:, ts:ts + tl],
                            wp[ob][:, o:o + 128],
                            ot_sb[ob][:, ts:ts + tl],
                            start=(ob == 0), stop=(ob == CB - 1))
                yt = ypool.tile([128, chunk_t], F32, tag="y")
                nc.scalar.activation(yt[:], pt[:], AF.Identity,
                                     bias=pb[:, opb:opb + 1])
                nc.sync.dma_start(yT_d[o:o + 128, t0:t0 + chunk_t], yt[:])

    nc.compile()
    return nc


def _host_prep(x, qkv_w, q_bias, v_bias, rpb_table, proj_w, proj_b, rel_index,
               wpc=WPC):
    x = np.asarray(x, np.float32)
    ncores = x.shape[0] // wpc
    t_total = wpc * N
    xT = np.ascontiguousarray(
        x.reshape(ncores, t_total, C).transpose(0, 2, 1))
    qkvwT = np.ascontiguousarray(np.asarray(qkv_w, np.float32).T)
    projwT = np.ascontiguousarray(np.asarray(proj_w, np.float32).T)
    qbT = np.ascontiguousarray(
        np.asarray(q_bias, np.float32).reshape(CB, 128).T)
    pbT = np.ascontiguousarray(
        np.asarray(proj_b, np.float32).reshape(CB, 128).T)
    vb = np.asarray(v_bias, np.float32).reshape(1, C)
    rel = np.asarray(rel_index).reshape(N, N)
    rpb = np.asarray(rpb_table, np.float32)[rel]          # [n, m, H]
    erp_full = np.exp(rpb).transpose(2, 1, 0)                 # [H, m, n]
    erpT = np.zeros((H, 128, 2 * N), np.float32)
    erpT[:, :, :N] = erp_full[:, 0:128, :]
    erpT[:, 0:68, N:] = erp_full[:, 128:196, :]
    erpT = erpT.astype(ml_dtypes.bfloat16)
    return xT, qkvwT, projwT, qbT, pbT, vb, erpT


def kernel(x, qkv_w, q_bias, v_bias, rpb_table, proj_w, proj_b, rel_index,
           num_heads=12, _trace=False):
    xT, qkvwT, projwT, qbT, pbT, vb, erpT = _host_prep(
        x, qkv_w, q_bias, v_bias, rpb_table, proj_w, proj_b, rel_index)
    if _trace:
        _install_ntff_hook()
    nc = _NC_CACHE.get("nc")
    if nc is None:
        nc = _build_nc()
        _NC_CACHE["nc"] = nc
    in_maps = [
        {"xT": np.ascontiguousarray(xT[c]), "qkvwT": qkvwT, "projwT": projwT,
         "qbT": qbT, "pbT": pbT, "vb": vb, "erpT": erpT,
         "onesrow": np.ones((1, 128), np.float32)}
        for c in range(NCORES)
    ]
    res = run_bass_kernel_spmd(nc, in_maps, core_ids=list(range(NCORES)),
                               trace=_trace)
    yT = np.stack([res.results[c]["yT"] for c in range(NCORES)])
    out = np.ascontiguousarray(yT.transpose(0, 2, 1)).reshape(B, N, C)
    if _trace:
        kernel._last_exec_time_ns = res.exec_time_ns
        kernel._last_results = res
    return out.astype(np.float32)
